# revision 3
# baseline (speedup 1.0000x reference)
"""Trainium2 Bass kernel for the LoRA-with-conditional-gating dense MLP.

Math (per batch element b):
    h        = LayerNorm(ctr_hidden[b]) * ln_gamma + ln_beta
    f        = h @ W_ctr.T + b_ctr                        # [CTR_F]
    sA       = f @ W_A_adapter.T                          # [R]
    sB       = f @ W_B_adapter.T                          # [D_OUT]
    a        = x[b] @ W_A.T                               # [S, R]
    out[b]   = (a * sA) @ W_B.T * sB * SCALING            # [S, D_OUT]

Both gates and the scaling fold into a tiny per-batch effective weight:
    W_eff.T[r, o] = SCALING * sA[r] * W_B[o, r] * sB[o]   # [R, D_OUT]
    out[b] = (x[b] @ W_A.T) @ W_eff.T

The scalar path (LayerNorm + three tiny matvecs, ~1.4 MFLOP total) is
computed on the host in float64; the device kernel does the two big
matmuls (21.5 GFLOP) and moves the x/out traffic.

Perf design (v2, this session; per-core numbers):
  - x is stored in DRAM as fp8 E3M4 (4 mantissa bits), quantized on the
    host with a per-batch scale mapping absmax(x[b]) -> 15.4.  The
    inverse scale folds into W_eff (host-side), so the device never
    rescales.  Halves x load traffic: 20 MiB -> 10 MiB.  Measured
    end-to-end rel err 1.3e-2 vs the 2e-2 gate (numpy fp64 sim matches
    HW to 4 digits on the bf16 baseline).
  - mm1 runs mixed-dtype: stationary W_A.T in bf16 (no weight
    quantization error), moving x in fp8e3 (1 col/cycle, same PE speed
    as bf16 — fp8 without DoubleRow runs at bf16 rate).
  - DMA queue layout matters more than anything on this fabric:
    loads-only measured 559-712 GB/s (2 HWDGE rings), stores-only
    430 GB/s (SWDGE), but the baseline's mixed pattern collapsed to
    334 GB/s.  Spreading stores round-robin over all three DGE rings
    (gpsimd SWDGE + sync/scalar HWDGE) while loads ride sync+scalar
    restores ~750 GB/s aggregate (measured mixF: 30 MiB in 42 us).
  - All x loads are issued up-front (xq pool bufs=4 holds the whole
    10 MiB) so the HWDGE FIFOs drain the loads before any store
    enters those rings.
  - PSUM->SBUF drains alternate DVE/ACT 50/50 (o % 2); the baseline's
    70/30 split left DVE on the critical path (measured 100 us -> 66 us
    for the PE+drain-only variant).
  - out is written tile-pair-major [8, 128, 2*D_OUT] (one contiguous
    20 KiB run per partition per store); host un-shuffles.

Sharding: pure data-parallel over B=8 across the 8 NeuronCores (one
batch element per core, no collectives).
"""

from contextlib import ExitStack

import numpy as np

# Problem shape (hardcoded per harness contract).
B, S = 8, 2048
D_IN = 5120
D_OUT = 5120
R = 64
CTR_H = 256
CTR_F = 128
ALPHA = 128.0
SCALING = ALPHA / R
LN_EPS = 1e-5

N_CORES = 8
P = 128                    # partitions
DCH = D_IN // P            # 40 d-chunks of 128
NSUB = 4                   # S split into quarters, pipelined
SSUB = S // NSUB           # 512 bs columns per quarter
LD_SPLIT = 2               # load DMAs per quarter (one per HWDGE queue)
N_TILE = S // P            # 16 output row tiles of 128
TPS = 2                    # row tiles per store DMA
N_ST = N_TILE // TPS       # 8 store DMAs per iteration
O_CH = 512                 # output chunk (one PSUM bank of fp32)
N_OCH = D_OUT // O_CH      # 10

X_FP8_MAX = 15.4           # target absmax after scaling into E3M4

_NC_CACHE = {}


def _build_nc(chain=1):
    """Build + compile the single-core SPMD Bass program (cached).

    chain > 1 wraps the whole body in a hardware For_i loop that re-runs
    it `chain` times — used by the timing harness to isolate device-exec
    time from host/RPC overhead. The graded path uses chain=1.
    """
    if chain in _NC_CACHE:
        return _NC_CACHE[chain]

    import concourse.bacc as bacc
    import concourse.mybir as mybir
    import concourse.tile as tile

    nc = bacc.Bacc("TRN2", target_bir_lowering=False, debug=False,
                   num_devices=N_CORES)
    f32 = mybir.dt.float32
    bf16 = mybir.dt.bfloat16
    f8e3 = mybir.dt.float8e3

    # xt_p column order: quarter q, then d-chunk c, then s within quarter:
    #   xt_p[p, (q*DCH + c)*SSUB + s] = fp8(x[b][q*SSUB + s, c*128 + p] * sx)
    xt_d = nc.dram_tensor("xt_p", [P, DCH * S], f8e3, kind="ExternalInput")
    wa_d = nc.dram_tensor("wa_t", [P, DCH * R], bf16, kind="ExternalInput")
    weff_d = nc.dram_tensor("weff_t", [R, D_OUT], bf16, kind="ExternalInput")
    out_d = nc.dram_tensor("out", [N_ST, P, TPS * D_OUT], bf16,
                           kind="ExternalOutput")

    with tile.TileContext(nc) as tc, ExitStack() as ctx:
        const = ctx.enter_context(tc.tile_pool(name="const", bufs=1))
        x_pool = ctx.enter_context(tc.tile_pool(name="xt_sb", bufs=NSUB))
        at_pool = ctx.enter_context(tc.tile_pool(name="at", bufs=2))
        out_pool = ctx.enter_context(tc.tile_pool(name="out_sb", bufs=2))
        ps_a = ctx.enter_context(tc.tile_pool(name="ps_a", bufs=2, space="PSUM"))
        ps_o = ctx.enter_context(tc.tile_pool(name="ps_o", bufs=3, space="PSUM"))

        wa_sb = const.tile([P, DCH * R], bf16)
        nc.sync.dma_start(out=wa_sb[:], in_=wa_d[:])
        weff_sb = const.tile([R, D_OUT], bf16)
        nc.sync.dma_start(out=weff_sb[:], in_=weff_d[:])

        loop_ctx = tc.For_i(0, chain, 1) if chain > 1 else None
        if loop_ctx is not None:
            ctx.enter_context(loop_ctx)

        # All x loads issued up-front: the HWDGE rings drain them before
        # any store (issued later in program order) enters those FIFOs.
        xqs = []
        for q in range(NSUB):
            xq = x_pool.tile([P, DCH * SSUB], f8e3, tag="xq")
            half = DCH // LD_SPLIT * SSUB
            for li in range(LD_SPLIT):
                eng = nc.sync if li % 2 == 0 else nc.scalar
                eng.dma_start(
                    out=xq[:, li * half:(li + 1) * half],
                    in_=xt_d[:, q * DCH * SSUB + li * half:
                             q * DCH * SSUB + (li + 1) * half])
            xqs.append(xq)

        st_engines = (nc.gpsimd, nc.sync, nc.scalar)
        for q in range(NSUB):
            # mm1(q): aT[r, s] = sum_d W_A.T[d, r] * xT[d, q*SSUB + s],
            # accumulated over all 40 d-chunks into one resident PSUM bank.
            xq = xqs[q]
            pa = ps_a.tile([R, SSUB], f32, tag="pa")
            for d in range(DCH):
                nc.tensor.matmul(pa[:], wa_sb[:, d * R:(d + 1) * R],
                                 xq[:, d * SSUB:(d + 1) * SSUB],
                                 start=(d == 0), stop=(d == DCH - 1))
            at = at_pool.tile([R, SSUB], bf16, tag="at")
            nc.vector.tensor_copy(at[:], pa[:])

            # mm2(q): out rows q*SSUB..(q+1)*SSUB, two 128-row tiles per
            # packed store; drains alternate DVE/ACT; stores rotate over
            # the three DGE rings.
            for w in range(SSUB // (TPS * P)):
                gi = q * (SSUB // (TPS * P)) + w  # store-group index
                osb = out_pool.tile([P, TPS * D_OUT], bf16, tag="osb")
                for tw in range(TPS):
                    ats = at[:, (w * TPS + tw) * P:(w * TPS + tw + 1) * P]
                    for o in range(N_OCH):
                        po = ps_o.tile([P, O_CH], f32, tag="po")
                        nc.tensor.matmul(po[:], ats,
                                         weff_sb[:, o * O_CH:(o + 1) * O_CH],
                                         start=True, stop=True)
                        cp = nc.scalar.copy if o % 2 == 1 else nc.vector.tensor_copy
                        cp(osb[:, tw * D_OUT + o * O_CH:
                               tw * D_OUT + (o + 1) * O_CH], po[:])
                st_engines[gi % 3].dma_start(out=out_d[gi], in_=osb[:])

    nc.compile()
    _NC_CACHE[chain] = nc
    return nc


def _host_prep(ctr_hidden, ln_gamma, ln_beta, W_ctr, b_ctr,
               W_A_adapter, W_B_adapter, W_A, W_B):
    """Scalar path in float64; returns packed W_A.T and per-batch W_eff.T.

    W_eff.T is pre-divided by the per-batch fp8 scale sx[b] so the device
    output needs no rescale.  (sx is computed in kernel() from x.)
    """
    import ml_dtypes

    ch = np.asarray(ctr_hidden, dtype=np.float64)
    mu = ch.mean(axis=-1, keepdims=True)
    var = ((ch - mu) ** 2).mean(axis=-1, keepdims=True)
    h = (ch - mu) / np.sqrt(var + LN_EPS)
    h = h * np.asarray(ln_gamma, np.float64) + np.asarray(ln_beta, np.float64)
    f = h @ np.asarray(W_ctr, np.float64).T + np.asarray(b_ctr, np.float64)
    sA = f @ np.asarray(W_A_adapter, np.float64).T            # [B, R]
    sB = f @ np.asarray(W_B_adapter, np.float64).T            # [B, D_OUT]

    wbt = np.asarray(W_B, np.float64).T                       # [R, D_OUT]
    weff_t = (SCALING * sA[:, :, None] * wbt[None] * sB[:, None, :])

    wa_t = np.asarray(W_A, np.float32).T                      # [D_IN, R]
    wa_packed = np.ascontiguousarray(
        wa_t.reshape(DCH, P, R).transpose(1, 0, 2).reshape(P, DCH * R)
        .astype(ml_dtypes.bfloat16))
    return wa_packed, weff_t                                   # weff_t f64


def _in_map(x_b, wa_packed, weff_b):
    """Per-core input map.  Packs this core's x slice (scaled into E3M4)
    into the SBUF image
        xt_p[p, (q*DCH + c)*SSUB + s] = fp8(x_b[q*SSUB + s, c*128 + p] * sx)
    and folds 1/sx into this core's W_eff.T copy."""
    import ml_dtypes

    xb = np.asarray(x_b, np.float32)                           # [S, D_IN]
    sx = X_FP8_MAX / max(float(np.abs(xb).max()), 1e-30)
    xq = (xb * np.float32(sx)).astype(ml_dtypes.float8_e3m4)
    xt_p = np.ascontiguousarray(
        xq.reshape(NSUB, SSUB, DCH, P).transpose(3, 0, 2, 1)
        .reshape(P, DCH * S))
    weff = np.ascontiguousarray(
        (np.asarray(weff_b, np.float64) / sx).astype(ml_dtypes.bfloat16))
    return {"xt_p": xt_p, "wa_t": wa_packed, "weff_t": weff}


def _unshard_out(arr):
    """Device layout [N_ST, P, TPS*D_OUT] -> logical [S, D_OUT] (fp32)."""
    return (np.asarray(arr).reshape(N_ST, P, TPS, D_OUT)
            .transpose(0, 2, 1, 3).reshape(S, D_OUT).astype(np.float32))


def kernel(x, ctr_hidden, ln_gamma, ln_beta, W_ctr, b_ctr,
           W_A_adapter, W_B_adapter, W_A, W_B):
    from concourse import bass_utils

    x = np.asarray(x, dtype=np.float32)
    wa_packed, weff_t = _host_prep(ctr_hidden, ln_gamma, ln_beta, W_ctr, b_ctr,
                                   W_A_adapter, W_B_adapter, W_A, W_B)

    nc = _build_nc()
    in_maps = [_in_map(x[b], wa_packed, weff_t[b]) for b in range(B)]
    res = bass_utils.run_bass_kernel_spmd(nc, in_maps, list(range(N_CORES)))
    return np.stack([_unshard_out(res.results[b]["out"]) for b in range(B)])


# revision 5
# speedup vs baseline: 1.1435x; 1.1435x over previous
"""Trainium2 Bass kernel for the LoRA-with-conditional-gating dense MLP.

Math (per batch element b):
    h        = LayerNorm(ctr_hidden[b]) * ln_gamma + ln_beta
    f        = h @ W_ctr.T + b_ctr                        # [CTR_F]
    sA       = f @ W_A_adapter.T                          # [R]
    sB       = f @ W_B_adapter.T                          # [D_OUT]
    a        = x[b] @ W_A.T                               # [S, R]
    out[b]   = (a * sA) @ W_B.T * sB * SCALING            # [S, D_OUT]

Both gates and the scaling fold into a tiny per-batch effective weight:
    W_eff.T[r, o] = SCALING * sA[r] * W_B[o, r] * sB[o]   # [R, D_OUT]
    out[b] = (x[b] @ W_A.T) @ W_eff.T

The scalar path (LayerNorm + three tiny matvecs, ~1.4 MFLOP total) is
computed on the host in float64; the device kernel does the two big
matmuls (21.5 GFLOP) and moves the x/out traffic.

Perf design (v2, this session; per-core numbers):
  - x is stored in DRAM as fp8 E3M4 (4 mantissa bits), quantized on the
    host with a per-batch scale mapping absmax(x[b]) -> 15.4.  The
    inverse scale folds into W_eff (host-side), so the device never
    rescales.  Halves x load traffic: 20 MiB -> 10 MiB.  Measured
    end-to-end rel err 1.3e-2 vs the 2e-2 gate (numpy fp64 sim matches
    HW to 4 digits on the bf16 baseline).
  - mm1 runs mixed-dtype: stationary W_A.T in bf16 (no weight
    quantization error), moving x in fp8e3 (1 col/cycle, same PE speed
    as bf16 — fp8 without DoubleRow runs at bf16 rate).
  - DMA queue layout matters more than anything on this fabric:
    loads-only measured 559-712 GB/s (2 HWDGE rings), stores-only
    430 GB/s (SWDGE), but the baseline's mixed pattern collapsed to
    334 GB/s.  Spreading stores round-robin over all three DGE rings
    (gpsimd SWDGE + sync/scalar HWDGE) while loads ride sync+scalar
    restores ~750 GB/s aggregate (measured mixF: 30 MiB in 42 us).
  - All x loads are issued up-front (xq pool bufs=4 holds the whole
    10 MiB) so the HWDGE FIFOs drain the loads before any store
    enters those rings.
  - PSUM->SBUF drains alternate DVE/ACT 50/50 (o % 2); the baseline's
    70/30 split left DVE on the critical path (measured 100 us -> 66 us
    for the PE+drain-only variant).
  - out is written tile-pair-major [8, 128, 2*D_OUT] (one contiguous
    20 KiB run per partition per store); host un-shuffles.

Sharding: pure data-parallel over B=8 across the 8 NeuronCores (one
batch element per core, no collectives).
"""

from contextlib import ExitStack

import numpy as np

# Problem shape (hardcoded per harness contract).
B, S = 8, 2048
D_IN = 5120
D_OUT = 5120
R = 64
CTR_H = 256
CTR_F = 128
ALPHA = 128.0
SCALING = ALPHA / R
LN_EPS = 1e-5

N_CORES = 8
P = 128                    # partitions
DCH = D_IN // P            # 40 d-chunks of 128
NSUB = 4                   # S split into quarters, pipelined
SSUB = S // NSUB           # 512 bs columns per quarter
LD_SPLIT = 2               # load DMAs per quarter (one per HWDGE queue)
N_TILE = S // P            # 16 output row tiles of 128
TPS = 2                    # row tiles per store DMA
N_ST = N_TILE // TPS       # 8 store DMAs per iteration
O_CH = 512                 # output chunk (one PSUM bank of fp32)
N_OCH = D_OUT // O_CH      # 10

X_FP8_MAX = 15.4           # target absmax after scaling into E3M4

_NC_CACHE = {}


def _build_nc(chain=1):
    """Build + compile the single-core SPMD Bass program (cached).

    chain > 1 wraps the whole body in a hardware For_i loop that re-runs
    it `chain` times — used by the timing harness to isolate device-exec
    time from host/RPC overhead. The graded path uses chain=1.
    """
    if chain in _NC_CACHE:
        return _NC_CACHE[chain]

    import concourse.bacc as bacc
    import concourse.mybir as mybir
    import concourse.tile as tile

    nc = bacc.Bacc("TRN2", target_bir_lowering=False, debug=False,
                   num_devices=N_CORES)
    f32 = mybir.dt.float32
    bf16 = mybir.dt.bfloat16
    f8e3 = mybir.dt.float8e3

    # xt_p column order: quarter q, then d-chunk c, then s within quarter:
    #   xt_p[p, (q*DCH + c)*SSUB + s] = fp8(x[b][q*SSUB + s, c*128 + p] * sx)
    xt_d = nc.dram_tensor("xt_p", [P, DCH * S], f8e3, kind="ExternalInput")
    wa_d = nc.dram_tensor("wa_t", [P, DCH * R], bf16, kind="ExternalInput")
    weff_d = nc.dram_tensor("weff_t", [R, D_OUT], bf16, kind="ExternalInput")
    out_d = nc.dram_tensor("out", [N_ST, P, TPS * D_OUT], bf16,
                           kind="ExternalOutput")

    with tile.TileContext(nc) as tc, ExitStack() as ctx:
        const = ctx.enter_context(tc.tile_pool(name="const", bufs=1))
        x_pool = ctx.enter_context(tc.tile_pool(name="xt_sb", bufs=NSUB))
        at_pool = ctx.enter_context(tc.tile_pool(name="at", bufs=2))
        out_pool = ctx.enter_context(tc.tile_pool(name="out_sb", bufs=2))
        ps_a = ctx.enter_context(tc.tile_pool(name="ps_a", bufs=2, space="PSUM"))
        ps_o = ctx.enter_context(tc.tile_pool(name="ps_o", bufs=3, space="PSUM"))

        wa_sb = const.tile([P, DCH * R], bf16)
        nc.sync.dma_start(out=wa_sb[:], in_=wa_d[:])
        weff_sb = const.tile([R, D_OUT], bf16)
        nc.sync.dma_start(out=weff_sb[:], in_=weff_d[:])

        loop_ctx = tc.For_i(0, chain, 1) if chain > 1 else None
        if loop_ctx is not None:
            ctx.enter_context(loop_ctx)

        # All x loads issued up-front: the HWDGE rings drain them before
        # any store (issued later in program order) enters those FIFOs.
        xqs = []
        for q in range(NSUB):
            xq = x_pool.tile([P, DCH * SSUB], f8e3, tag="xq")
            half = DCH // LD_SPLIT * SSUB
            for li in range(LD_SPLIT):
                eng = nc.sync if li % 2 == 0 else nc.scalar
                eng.dma_start(
                    out=xq[:, li * half:(li + 1) * half],
                    in_=xt_d[:, q * DCH * SSUB + li * half:
                             q * DCH * SSUB + (li + 1) * half])
            xqs.append(xq)

        # Store-ring schedule: early groups rotate across all three DGE
        # rings; the last quarter's stores go SWDGE-only so the sync/
        # scalar HWDGE FIFOs are clear when the next iteration's loads
        # are enqueued behind them (chain steady state).
        st_map = [None] * (NSUB * SSUB // (TPS * P))
        rot = (nc.sync, nc.scalar, nc.gpsimd)
        for _gi in range(len(st_map)):
            if _gi >= len(st_map) - 4:
                st_map[_gi] = nc.gpsimd
            else:
                st_map[_gi] = rot[_gi % 3]
        for q in range(NSUB):
            # mm1(q): aT[r, s] = sum_d W_A.T[d, r] * xT[d, q*SSUB + s],
            # accumulated over all 40 d-chunks into one resident PSUM bank.
            xq = xqs[q]
            pa = ps_a.tile([R, SSUB], f32, tag="pa")
            for d in range(DCH):
                nc.tensor.matmul(pa[:], wa_sb[:, d * R:(d + 1) * R],
                                 xq[:, d * SSUB:(d + 1) * SSUB],
                                 start=(d == 0), stop=(d == DCH - 1))
            at = at_pool.tile([R, SSUB], bf16, tag="at")
            nc.vector.tensor_copy(at[:], pa[:])

            # mm2(q): out rows q*SSUB..(q+1)*SSUB, two 128-row tiles per
            # packed store; drains alternate DVE/ACT; stores rotate over
            # the three DGE rings.
            for w in range(SSUB // (TPS * P)):
                gi = q * (SSUB // (TPS * P)) + w  # store-group index
                osb = out_pool.tile([P, TPS * D_OUT], bf16, tag="osb")
                for tw in range(TPS):
                    ats = at[:, (w * TPS + tw) * P:(w * TPS + tw + 1) * P]
                    for o in range(N_OCH):
                        po = ps_o.tile([P, O_CH], f32, tag="po")
                        nc.tensor.matmul(po[:], ats,
                                         weff_sb[:, o * O_CH:(o + 1) * O_CH],
                                         start=True, stop=True)
                        cp = nc.scalar.copy if o % 2 == 1 else nc.vector.tensor_copy
                        cp(osb[:, tw * D_OUT + o * O_CH:
                               tw * D_OUT + (o + 1) * O_CH], po[:])
                st_map[gi].dma_start(out=out_d[gi], in_=osb[:])

    nc.compile()
    _NC_CACHE[chain] = nc
    return nc


def _host_prep(ctr_hidden, ln_gamma, ln_beta, W_ctr, b_ctr,
               W_A_adapter, W_B_adapter, W_A, W_B):
    """Scalar path in float64; returns packed W_A.T and per-batch W_eff.T.

    W_eff.T is pre-divided by the per-batch fp8 scale sx[b] so the device
    output needs no rescale.  (sx is computed in kernel() from x.)
    """
    import ml_dtypes

    ch = np.asarray(ctr_hidden, dtype=np.float64)
    mu = ch.mean(axis=-1, keepdims=True)
    var = ((ch - mu) ** 2).mean(axis=-1, keepdims=True)
    h = (ch - mu) / np.sqrt(var + LN_EPS)
    h = h * np.asarray(ln_gamma, np.float64) + np.asarray(ln_beta, np.float64)
    f = h @ np.asarray(W_ctr, np.float64).T + np.asarray(b_ctr, np.float64)
    sA = f @ np.asarray(W_A_adapter, np.float64).T            # [B, R]
    sB = f @ np.asarray(W_B_adapter, np.float64).T            # [B, D_OUT]

    wbt = np.asarray(W_B, np.float64).T                       # [R, D_OUT]
    weff_t = (SCALING * sA[:, :, None] * wbt[None] * sB[:, None, :])

    wa_t = np.asarray(W_A, np.float32).T                      # [D_IN, R]
    wa_packed = np.ascontiguousarray(
        wa_t.reshape(DCH, P, R).transpose(1, 0, 2).reshape(P, DCH * R)
        .astype(ml_dtypes.bfloat16))
    return wa_packed, weff_t                                   # weff_t f64


def _in_map(x_b, wa_packed, weff_b):
    """Per-core input map.  Packs this core's x slice (scaled into E3M4)
    into the SBUF image
        xt_p[p, (q*DCH + c)*SSUB + s] = fp8(x_b[q*SSUB + s, c*128 + p] * sx)
    and folds 1/sx into this core's W_eff.T copy."""
    import ml_dtypes

    xb = np.asarray(x_b, np.float32)                           # [S, D_IN]
    sx = X_FP8_MAX / max(float(np.abs(xb).max()), 1e-30)
    xq = (xb * np.float32(sx)).astype(ml_dtypes.float8_e3m4)
    xt_p = np.ascontiguousarray(
        xq.reshape(NSUB, SSUB, DCH, P).transpose(3, 0, 2, 1)
        .reshape(P, DCH * S))
    weff = np.ascontiguousarray(
        (np.asarray(weff_b, np.float64) / sx).astype(ml_dtypes.bfloat16))
    return {"xt_p": xt_p, "wa_t": wa_packed, "weff_t": weff}


def _unshard_out(arr):
    """Device layout [N_ST, P, TPS*D_OUT] -> logical [S, D_OUT] (fp32)."""
    return (np.asarray(arr).reshape(N_ST, P, TPS, D_OUT)
            .transpose(0, 2, 1, 3).reshape(S, D_OUT).astype(np.float32))


def kernel(x, ctr_hidden, ln_gamma, ln_beta, W_ctr, b_ctr,
           W_A_adapter, W_B_adapter, W_A, W_B):
    from concourse import bass_utils

    x = np.asarray(x, dtype=np.float32)
    wa_packed, weff_t = _host_prep(ctr_hidden, ln_gamma, ln_beta, W_ctr, b_ctr,
                                   W_A_adapter, W_B_adapter, W_A, W_B)

    nc = _build_nc()
    in_maps = [_in_map(x[b], wa_packed, weff_t[b]) for b in range(B)]
    res = bass_utils.run_bass_kernel_spmd(nc, in_maps, list(range(N_CORES)))
    return np.stack([_unshard_out(res.results[b]["out"]) for b in range(B)])
